# revision 17
# baseline (speedup 1.0000x reference)
"""Trainium2 Bass kernel for quantized Conv1D forward:
    y = x @ (w_q * scale) + bias
  x:     [4, 2048, 4096] f32
  w_q:   [4096, 16384] int32 (values in [-127, 127])
  scale: [16384] f32
  bias:  [16384] f32
  y:     [4, 2048, 16384] f32

Sharding: column-parallel over out_features across 8 cores (N=2048 each);
x replicated. Each core computes y_shard = x @ (w_q_shard * scale_shard)
+ bias_shard independently (no collectives); host concatenates shards.

Device strategy:
  - Single 16-bit matmul pass (MM_DT, default bf16): x rounded to bf16 and
    scale folded into the weights on the host (wb = bf16(w_q * scale),
    range ~[1e-4, 1.29]).  Accumulation is fp32 in PSUM; total rel err
    ~2.3e-3, ~9x under the 2e-2 gate.  bf16 measured ~10% faster than
    IEEE fp16 on HW (position-controlled wall(reps=7) duels) — consistent
    with the FWL fast-weight-load path covering bf16/fp8 but not fp16.
    Set MM_DT="fp16" for ~3e-4 rel err, X_SPLIT=True for the 2-pass
    ~fp32-exact hi/lo path.
  - Only bias remains as a device-side epilogue (one DVE add per chunk).
  - The 16-bit weight shard [4096, 2048] stays fully resident in SBUF
    (128 KB/partition); x tiles stream through; PE runs back-to-back
    matmuls (stationary = x^T tile, moving = w rows, N=512 per PSUM bank).
"""

import numpy as np

import concourse.bass as bass
import concourse.mybir as mybir
import concourse.tile as tile
from concourse import bacc
from concourse.bass import ts
from concourse.bass_utils import run_bass_kernel_spmd

P = 128
N_CORES = 8

# numerics strategy: True = x split into fp16 hi+lo (2 matmul passes,
# ~fp32-exact); False = single fp16 pass for x (~4e-4 rel err, 2x faster)
X_SPLIT = False
# True: scale is multiplied into the fp16 weights on the host; no scale
# input / on-device dequant mul (requires X_SPLIT=False for accuracy headroom)
FOLD_SCALE = True
# matmul dtype: "bf16" (~2.3e-3 rel err, ~10% faster: FWL weight-load path
# covers bf16/fp8 but not IEEE fp16) or "fp16" (~3e-4 rel err)
MM_DT = "bf16"


def _np_mm_dt():
    if MM_DT == "bf16":
        import ml_dtypes

        return ml_dtypes.bfloat16
    return np.float16


def build_nc(T, K, N, x_split=X_SPLIT, n_free=512, reps=1,
             x_bufs=2, o_bufs=2, p_bufs=None, x_dma_split=1, swap_loop=False,
             mm_dt=None, w_split=False, y_dma_split=1, n_sub=1, sub_order="A",
             fold_scale=FOLD_SCALE,
             skip_ydma=False, dummy_x=False, skip_vec=False):
    """Build the per-core Bass program.

    DRAM I/O (per core):
      xh:    [TB, P, S, KB, Tt] fp16  packed x^T tiles (S=2 if split: hi,lo)
      wh:    [P, KB, N]         fp16  weight shard, k on partitions
      scale: [N] f32  (absent when fold_scale)
      bias:  [N] f32
      y:     [T, N] f32 out
    """
    KB = K // P
    TB = T // P
    Tt = P
    NB = N // n_free
    S = 2 if x_split else 1
    if mm_dt is None:
        mm_dt = MM_DT
    mdt = {"fp16": mybir.dt.float16, "bf16": mybir.dt.bfloat16,
           "fp32r": mybir.dt.float32r}[mm_dt]

    nc = bacc.Bacc("TRN2", target_bir_lowering=False, debug=False)

    xh = nc.dram_tensor("xh", [TB, P, S, KB, Tt], mdt, kind="ExternalInput")
    wh = nc.dram_tensor("wh", [P, KB, N], mdt, kind="ExternalInput")
    scale_h = None
    if not fold_scale:
        scale_h = nc.dram_tensor("scale", [N], mybir.dt.float32, kind="ExternalInput")
    bias_h = nc.dram_tensor("bias", [N], mybir.dt.float32, kind="ExternalInput")
    y_h = nc.dram_tensor("y", [T, N], mybir.dt.float32, kind="ExternalOutput")

    xh_ap = xh.ap()
    wh_ap = wh.ap()
    y_ap = y_h.ap().rearrange("(tb p) n -> tb p n", p=P)

    def bcast_ap(ap):
        # [N] dram vector -> [P, N] with step-0 partition dim for DMA broadcast
        return bass.AP(tensor=ap.tensor, offset=ap.offset, ap=[[0, P], *ap.ap])

    with tile.TileContext(nc) as tc:
        if p_bufs is None:
            p_bufs = 2 * NB
        with (
            tc.tile_pool(name="wpool", bufs=1) as wpool,
            tc.tile_pool(name="cpool", bufs=1) as cpool,
            tc.tile_pool(name="xpool", bufs=x_bufs) as xpool,
            tc.tile_pool(name="opool", bufs=o_bufs) as opool,
            tc.tile_pool(name="ppool", bufs=p_bufs, space="PSUM") as ppool,
        ):
            # resident weights: [P, KB, N]; per-kb tiles give per-slice deps
            if w_split:
                w_tiles = []
                for kb in range(KB):
                    wt = wpool.tile([P, N], mdt, name=f"w{kb}")
                    nc.sync.dma_start(out=wt[:], in_=wh_ap[:, kb])
                    w_tiles.append(wt)
                w_rhs = lambda kb, c0, w: w_tiles[kb][:, c0:c0 + w]
            else:
                w_res = wpool.tile([P, KB, N], mdt, name="w_res")
                for kb in range(KB):
                    nc.sync.dma_start(out=w_res[:, kb], in_=wh_ap[:, kb])
                w_rhs = lambda kb, c0, w: w_res[:, kb, c0:c0 + w]

            scale_b = None
            if not fold_scale:
                scale_b = cpool.tile([P, N], mybir.dt.float32, name="scale_b")
                nc.sync.dma_start(out=scale_b[:], in_=bcast_ap(scale_h.ap()))
            bias_b = cpool.tile([P, N], mybir.dt.float32, name="bias_b")
            nc.sync.dma_start(out=bias_b[:], in_=bcast_ap(bias_h.ap()))

            xt_shared = None
            if dummy_x:
                xt_shared = xpool.tile([P, S, KB, Tt], mdt, name="xt_shared")
                nc.sync.dma_start(out=xt_shared[:], in_=xh_ap[0])

            for tb in [t for _ in range(reps) for t in range(TB)]:
                if dummy_x:
                    xt = xt_shared
                else:
                    xt = xpool.tile([P, S, KB, Tt], mdt, tag="xt")
                    if x_dma_split == 1:
                        nc.sync.dma_start(out=xt[:], in_=xh_ap[tb])
                    else:
                        assert KB % x_dma_split == 0
                        c = KB // x_dma_split
                        for s in range(S):
                            for d in range(x_dma_split):
                                nc.sync.dma_start(
                                    out=xt[:, s, ts(d, c)],
                                    in_=xh_ap[tb, :, s, ts(d, c)],
                                )

                psums = [
                    ppool.tile([P, n_free], mybir.dt.float32, tag="acc", name=f"ps{nb}")
                    for nb in range(NB)
                ]
                # n_sub>1: split each PSUM bank into n_sub independent
                # accumulation regions (has_written is per-element), raising
                # stationary reuse from NB to NB*n_sub MMs per weight load
                nsl = NB * n_sub          # total slices
                w_free = n_free // n_sub  # moving free per MM
                if n_sub > 1:
                    if sub_order == "A":  # slice-major: cycle banks, then halves
                        sl_iter = [(b, h) for h in range(n_sub) for b in range(NB)]
                    else:  # "B" bank-major: both halves of a bank back-to-back
                        sl_iter = [(b, h) for b in range(NB) for h in range(n_sub)]
                    mm_iter = [
                        (kb, s, b, h)
                        for kb in range(KB)
                        for s in range(S)
                        for (b, h) in sl_iter
                    ]
                elif swap_loop:
                    mm_iter = [
                        (kb, s, nb, 0)
                        for nb in range(NB)
                        for kb in range(KB)
                        for s in range(S)
                    ]
                else:
                    mm_iter = [
                        (kb, s, nb, 0)
                        for kb in range(KB)
                        for s in range(S)
                        for nb in range(NB)
                    ]
                for kb, s, b, h in mm_iter:
                    nc.tensor.matmul(
                        psums[b][:, ts(h, w_free)],
                        lhsT=xt[:, s, kb, :],
                        rhs=w_rhs(kb, b * n_free + h * w_free, w_free),
                        start=(kb == 0 and s == 0 and h == 0),
                        stop=(kb == KB - 1 and s == S - 1),
                        skip_group_check=(n_sub > 1),
                    )

                if skip_vec:
                    continue
                out_sb = opool.tile([P, N], mybir.dt.float32, tag="out")
                for nb in range(NB):
                    if fold_scale:
                        nc.vector.tensor_add(
                            out=out_sb[:, ts(nb, n_free)],
                            in0=psums[nb][:],
                            in1=bias_b[:, ts(nb, n_free)],
                        )
                    else:
                        nc.vector.tensor_mul(
                            out=out_sb[:, ts(nb, n_free)],
                            in0=psums[nb][:],
                            in1=scale_b[:, ts(nb, n_free)],
                        )
                        nc.vector.tensor_add(
                            out=out_sb[:, ts(nb, n_free)],
                            in0=out_sb[:, ts(nb, n_free)],
                            in1=bias_b[:, ts(nb, n_free)],
                        )
                if skip_ydma:
                    continue
                if y_dma_split == 1:
                    nc.sync.dma_start(out=y_ap[tb], in_=out_sb[:])
                else:
                    c = N // y_dma_split
                    for d in range(y_dma_split):
                        nc.sync.dma_start(
                            out=y_ap[tb, :, ts(d, c)], in_=out_sb[:, ts(d, c)]
                        )

    nc.compile()
    return nc


def pack_x(x2d, T, K, x_split=X_SPLIT, np_dt=None):
    """[T, K] f32 -> [TB, P, S, KB, Tt] tiles of x^T (hi[, lo])."""
    TB, KB = T // P, K // P
    if np_dt is None:
        np_dt = _np_mm_dt()
    x_hi = x2d.astype(np_dt)
    # [T, K] -> [TB, Tt, KB, Pk] -> [TB, Pk, KB, Tt]
    def tilev(a):
        return np.ascontiguousarray(
            a.reshape(TB, P, KB, P).transpose(0, 3, 2, 1)
        )
    if not x_split:
        return tilev(x_hi)[:, :, None, :, :]
    x_lo = (x2d - x_hi.astype(np.float32)).astype(np_dt)
    out = np.empty((TB, P, 2, KB, P), dtype=np_dt)
    out[:, :, 0] = tilev(x_hi)
    out[:, :, 1] = tilev(x_lo)
    return out


def pack_w(w_shard, K, N, np_dt=None, scale=None):
    """[K, N] int -> [P, KB, N]; optionally folds per-column scale in."""
    KB = K // P
    if np_dt is None:
        np_dt = _np_mm_dt()
    w = w_shard.astype(np.float32)
    if scale is not None:
        w = w * scale[None, :].astype(np.float32)
    return np.ascontiguousarray(
        w.astype(np_dt).reshape(KB, P, N).transpose(1, 0, 2)
    )


_NC_CACHE = {}

# tuned on hardware: x DMA in 4 chunks/plane + two independent 256-wide
# accumulation regions per PSUM bank (stationary reused 8 MMs per load);
# beat the nb-outer/kb-inner N=512 order 4.34 vs 4.40 ms same-process
TUNED = dict(x_dma_split=4, n_sub=2, sub_order="B")


def _get_nc(T, K, N, x_split, fold_scale):
    key = (T, K, N, x_split, fold_scale)
    if key not in _NC_CACHE:
        _NC_CACHE[key] = build_nc(T, K, N, x_split=x_split,
                                  fold_scale=fold_scale, **TUNED)
    return _NC_CACHE[key]


def kernel(x, w_q, scale, bias):
    x = np.asarray(x)
    w_q = np.asarray(w_q)
    scale = np.asarray(scale, dtype=np.float32)
    bias = np.asarray(bias, dtype=np.float32)
    B, Sq, K = x.shape
    K2, D_OUT = w_q.shape
    assert K2 == K
    T = B * Sq
    N = D_OUT // N_CORES

    nc = _get_nc(T, K, N, X_SPLIT, FOLD_SCALE)

    xh = pack_x(np.ascontiguousarray(x.reshape(T, K)), T, K, X_SPLIT)
    in_maps = []
    for c in range(N_CORES):
        sl = slice(c * N, (c + 1) * N)
        m = {
            "xh": xh,
            "wh": pack_w(w_q[:, sl], K, N,
                         scale=scale[sl] if FOLD_SCALE else None),
            "bias": np.ascontiguousarray(bias[sl], dtype=np.float32),
        }
        if not FOLD_SCALE:
            m["scale"] = np.ascontiguousarray(scale[sl], dtype=np.float32)
        in_maps.append(m)

    res = run_bass_kernel_spmd(nc, in_maps, core_ids=list(range(N_CORES)))
    y = np.concatenate([r["y"] for r in res.results], axis=1)
    return y.reshape(B, Sq, D_OUT)

